# revision 7
# baseline (speedup 1.0000x reference)
"""Multi-head attention (B=2, N=2048, D=1024, H=16, d_k=d_v=64) on 8
TRN2 NeuronCores — v2: fp8e4m3 datapath with DoubleRow matmuls.

Sharding: data parallel over batch (2) x tensor parallel over head
groups (4 heads per core). Host sums the 4 partial projections per
batch and adds the residual.

v2 design vs baseline:
  - all projections (Q/K/V, output) and attn@V run as fp8e4m3
    DoubleRow matmuls (256-deep contraction per MM, ~1.77x PE rate).
  - exp is split across the Scalar engine (table exp -> fp8 dst) and
    the Vector engine (Schraudolph bit-trick: uint8 = round(
    log2e*s + B) bitcast as fp8e4m3). Both compute exp(s/8 - 3.8);
    the shift cancels in softmax and keeps fp8 in range.
  - softmax denominator comes from a ones-column in the fp8 V tile
    (row 64 of the attn@V psum); its reciprocal runs on 128 partitions
    after a [1,512]->[128,4] transpose DMA (was 3.3us/row on 1 lane).
  - the two head-group passes are interleaved (g2 alternates per query
    block) so ACT/DVE exp load and the projection tail spread evenly.
"""
import numpy as np

import concourse.bass as bass
import concourse.tile as tile
from concourse import mybir
from concourse.vector_clock import ScopedClock

f32 = mybir.dt.float32
bf16 = mybir.dt.bfloat16
fp8 = mybir.dt.float8e4
u8 = mybir.dt.uint8
DR = mybir.MatmulPerfMode.DoubleRow

B, N, D = 2, 2048, 1024
H, DK = 16, 64
HPC = 4           # heads per core
GCOLS = HPC * DK  # 256 weight columns per core
NCORES = 8
P = 128
NQB = 4           # query blocks of 512
NKB = 16          # key blocks of 128
NKBP = 8          # key block pairs (DoubleRow attn@V)
NC = 4            # d_model DoubleRow chunks of 256
NSB = 16          # token blocks of 128 for the projection
NVQ = 8           # interleaved (qb, g2) attention passes

EXP_SHIFT = 3.8
# uint8 Schraudolph: i = round(SCH_MULT*s_raw + SCH_ADD); bits are fp8e4m3
# of ~exp(s_raw/8 - EXP_SHIFT).  56 = 7(bias)*8; -0.344 centers the
# piecewise-linear interpolation error.
SCH_MULT = 0.125 * 8 * 1.4426950408889634
SCH_ADD = 56.0 - 8 * 1.4426950408889634 * EXP_SHIFT - 0.344

# kb slots whose exp runs on ACT (rest on DVE).  ACT is faster per element
# (1.21us vs 1.46us per [128,1024] tile measured), so it takes 11/16; 14/15
# kept on ACT so the DVE queue is free at the vq boundary for oc copies.
ACT_KBS = (0, 1, 3, 4, 6, 8, 9, 11, 13, 14, 15)

_cache = {}
_last_results = None


# ---------------------------------------------------------------------------
# Workarounds for this walrus build: max ONE sync wait per instruction.
# ---------------------------------------------------------------------------
_ws_counter = [0]


def _split_multi_waits(nc, limit=1):
    for f in nc.m.functions:
        for bb in f.blocks:
            new = []
            changed = False
            for inst in bb.instructions:
                si = inst.sync_info
                waits = list(si.on_wait) if si is not None and si.on_wait else []
                if len(waits) > limit:
                    changed = True
                    extra = waits[:-limit]
                    for i in range(0, len(extra), limit):
                        _ws_counter[0] += 1
                        nop = mybir.InstNoOp(
                            name=f"I-waitsplit-{_ws_counter[0]}", ins=[], outs=[]
                        )
                        nop.engine = inst.engine
                        nop.sync_info = mybir.SyncInfo(
                            on_wait=extra[i : i + limit], on_update=[]
                        )
                        new.append(nop)
                    si.on_wait = waits[-limit:]
                    inst.sync_info = si
                new.append(inst)
            if changed:
                bb.instructions = new


def _patched_drain_and_barrier(self, tick_clock, wait_clock):
    nc = self.nc
    drain_inst = nc.sync.drain()
    wait_clock.add_sem_waits(
        drain_inst.ins, ScopedClock({None: tick_clock.global_clock})
    )
    si = drain_inst.ins.sync_info
    if si is not None and si.on_wait is not None and len(si.on_wait) > 1:
        waits = list(si.on_wait)
        si.on_wait = waits[:1]
        drain_inst.ins.sync_info = si
        for i in range(1, len(waits)):
            extra = nc.sync.drain()
            esi = extra.ins.sync_info
            if esi is None:
                esi = mybir.SyncInfo(on_wait=[], on_update=[])
            esi.on_wait = waits[i : i + 1]
            extra.ins.sync_info = esi
    nc.all_engine_barrier()
    assert self.sems is not None
    popped = nc._tile_sem_poison_stack.pop()
    assert popped is self._sem_poison
    nc.clear_and_free_semaphores(list(self.sems.allocated().values()))
    nc.all_engine_barrier()


tile.TileContext._drain_and_barrier = _patched_drain_and_barrier


# ---------------------------------------------------------------------------
# Kernel build
# ---------------------------------------------------------------------------
def _build():
    nc = bass.Bass()
    # xti[p, c, t, n] = x[n, c*256 + t*128 + p]
    xti = nc.dram_tensor("xti", [P, NC, 2, N], fp8, kind="ExternalInput")
    # w(q/k/v)i[p, c, t, m] = W[c*256 + t*128 + p, cs][m]
    wqi = nc.dram_tensor("wqi", [P, NC, 2, GCOLS], fp8, kind="ExternalInput")
    wki = nc.dram_tensor("wki", [P, NC, 2, GCOLS], fp8, kind="ExternalInput")
    wvi = nc.dram_tensor("wvi", [P, NC, 2, GCOLS], fp8, kind="ExternalInput")
    # wpi[p, t, m] = Wp[cs][t*128 + p, m]
    wpi = nc.dram_tensor("wpi", [P, 2, D], fp8, kind="ExternalInput")
    pout = nc.dram_tensor("pout", [N, D], bf16, kind="ExternalOutput")

    with tile.TileContext(nc) as tc:
        import contextlib

        with contextlib.ExitStack() as ctx:
            sbX = ctx.enter_context(tc.tile_pool(name="sbX", bufs=1))
            sbW = ctx.enter_context(tc.tile_pool(name="sbW", bufs=1))
            sbQK = ctx.enter_context(tc.tile_pool(name="sbQK", bufs=1))
            sbV = ctx.enter_context(tc.tile_pool(name="sbV", bufs=1))
            sbO = ctx.enter_context(tc.tile_pool(name="sbO", bufs=1))
            sbA = ctx.enter_context(tc.tile_pool(name="sbA", bufs=4))
            sbR = ctx.enter_context(tc.tile_pool(name="sbR", bufs=4))
            sbP = ctx.enter_context(tc.tile_pool(name="sbP", bufs=3))
            drS = ctx.enter_context(tc.tile_pool(name="drS", bufs=4,
                                                 space="DRAM"))
            psA = ctx.enter_context(tc.tile_pool(name="psA", bufs=1,
                                                 space="PSUM"))
            psS = ctx.enter_context(tc.tile_pool(name="psS", bufs=2,
                                                 space="PSUM"))
            psO = ctx.enter_context(tc.tile_pool(name="psO", bufs=3,
                                                 space="PSUM"))

            # ---- loads & constants ---------------------------------------
            # weights first (small; chains need them with the first x slice),
            # then x by query-block so the qb=0 slice lands early and the
            # prologue q/k/v chains start ~8us before the full load is done.
            wq_t = sbW.tile([P, NC, 2, GCOLS], fp8, tag="wq", name="wq")
            wk_t = sbW.tile([P, NC, 2, GCOLS], fp8, tag="wk", name="wk")
            wv_t = sbW.tile([P, NC, 2, GCOLS], fp8, tag="wv", name="wv")
            wp_t = sbW.tile([P, 2, D], fp8, tag="wp", name="wp")
            xt = sbX.tile([P, NC, 2, N], fp8, tag="xt", name="xt")
            # DMA queues drain FIFO: order by when the prologue needs data.
            # The prologue q00/k00/v0/v1 chains and vq0's first half only
            # touch token/key columns 0:1024, so load x in column halves
            # with the A-half first — chains start ~6us earlier.
            nc.sync.dma_start(out=wq_t[:], in_=wqi[:])
            nc.sync.dma_start(out=xt[:, 0, :, 0:1024], in_=xti[:, 0, :, 0:1024])
            nc.sync.dma_start(out=wk_t[:], in_=wki[:])
            nc.sync.dma_start(out=wv_t[:], in_=wvi[:])
            for c in range(1, NC):
                nc.sync.dma_start(out=xt[:, c, :, 0:1024],
                                  in_=xti[:, c, :, 0:1024])
            for c in range(NC):
                nc.sync.dma_start(out=xt[:, c, :, 1024:2048],
                                  in_=xti[:, c, :, 1024:2048])
            nc.sync.dma_start(out=wp_t[:], in_=wpi[:])

            bias_t = sbW.tile([P, 1], f32, tag="bias", name="bias")
            nc.vector.memset(bias_t[:], -EXP_SHIFT)

            # Pre-warm the exp table (~2.7us ACT table load) during loads.
            warm = sbR.tile([1, 1], f32, tag="warm")
            nc.scalar.activation(
                warm[:], bias_t[0:1, :],
                mybir.ActivationFunctionType.Exp,
            )

            qT = [sbQK.tile([P, N], bf16, tag=f"qT{g2}", name=f"qT{g2}")
                  for g2 in range(2)]
            kT = [sbQK.tile([P, N], bf16, tag=f"kT{g2}", name=f"kT{g2}")
                  for g2 in range(2)]
            # vaug[k, kbp, h4, t, 0:64] = v; col 64 = 1.0 (denominator)
            vaug = sbV.tile([P, NKBP, HPC, 2, 80], fp8, tag="vaug",
                            name="vaug")
            nc.vector.memset(vaug[:, :, :, :, 64:65], 1.0)
            # outTi[p, g2, n]: p = hl*64 + dv -> Wp row g2*128 + p
            outTi = sbO.tile([P, 2, N], fp8, tag="outTi", name="outTi")

            # ---- chain thunk builders ------------------------------------
            def _psS_acc(name):
                return psS.tile([P, 2, 512], f32, tag="s", name=name)[:, 0, :]

            def qk_chain_thunks(dstT, w_t, g2, qb, alloc=None):
                st = {}
                def mk(c):
                    def mm():
                        if c == 0:
                            if alloc is not None:
                                st["p"] = alloc(f"pqk{g2}_{qb}_{id(w_t)%97}")
                            else:
                                st["p"] = psA.tile([P, 512], f32, tag="pacc",
                                               name=f"pqk{g2}_{qb}_{id(w_t)%97}")
                        nc.tensor.matmul(
                            st["p"][:],
                            w_t[:, c, :, g2 * P : (g2 + 1) * P],
                            xt[:, c, :, qb * 512 : (qb + 1) * 512],
                            start=(c == 0), stop=(c == NC - 1),
                            perf_mode=DR,
                        )
                    return mm
                def evict():
                    nc.vector.tensor_copy(
                        dstT[g2][:, qb * 512 : (qb + 1) * 512], st["p"][:]
                    )
                return [mk(c) for c in range(NC)] + [evict]

            def v_chain_thunks(kb):
                st = {}
                def mk(c):
                    def mm():
                        if c == 0:
                            st["p"] = psA.tile([P, 512], f32, tag="pacc",
                                               name=f"pv{kb}")
                        nc.tensor.matmul(
                            st["p"][:, 0:GCOLS],
                            xt[:, c, :, kb * P : (kb + 1) * P],
                            wv_t[:, c, :, :],
                            start=(c == 0), stop=(c == NC - 1),
                            perf_mode=DR,
                        )
                    return mm
                def evict():
                    nc.vector.tensor_copy(
                        vaug[:, kb // 2, :, kb % 2, 0:64],
                        st["p"][:, 0:GCOLS].rearrange(
                            "p (h d) -> p h d", h=HPC),
                    )
                return [mk(c) for c in range(NC)] + [evict]

            def proj_thunks(sb, tail=False):
                st = {}
                ot = sbP.tile([P, D], bf16, tag="pout", name=f"ot{sb}")
                def mm(half):
                    if tail and half == 0:
                        # psS banks are dead in the tail; borrow them so the
                        # last projections pipeline instead of serializing
                        # on the single psA slot.
                        st[half] = _psS_acc(f"pp{sb}_{half}")
                    else:
                        st[half] = psA.tile([P, 512], f32, tag="pacc",
                                            name=f"pp{sb}_{half}")
                    nc.tensor.matmul(
                        st[half][:],
                        outTi[:, :, sb * P : (sb + 1) * P],
                        wp_t[:, :, half * 512 : (half + 1) * 512],
                        start=True, stop=True, perf_mode=DR,
                    )
                def ev0():
                    if tail:
                        nc.scalar.copy(ot[:, 0:512], st[0][:])
                    else:
                        nc.vector.tensor_copy(ot[:, 0:512], st[0][:])
                def ev1():
                    nc.vector.tensor_copy(ot[:, 512:1024], st[1][:])
                def dma():
                    nc.sync.dma_start(
                        out=pout[sb * P : (sb + 1) * P, :], in_=ot[:]
                    )
                return [lambda: mm(0), ev0, lambda: mm(1), ev1, dma]

            # ---- fill queue (requirement-gated chains) -------------------
            fillq = []
            fill_state = {"v": 1, "k0": 0, "k1": -1,
                          "q": {(0, 0)}}

            def fq_push(kind, idx, thunks):
                for i, t in enumerate(thunks):
                    fillq.append((kind, idx, i == len(thunks) - 1, t))

            def fq_pop():
                kind, idx, last, t = fillq.pop(0)
                t()
                if last:
                    if kind == "v":
                        fill_state["v"] = idx
                    elif kind == "k":
                        fill_state[f"k{idx[0]}"] = idx[1]
                    elif kind == "q":
                        fill_state["q"].add(idx)

            def k_done(g2, kbk):
                return fill_state[f"k{g2}"] >= kbk

            def drain_until(pred):
                while fillq and not pred():
                    fq_pop()

            side = []  # opportunistic queue (proj)

            def pull_side(slots_left):
                q = fillq if fillq else side
                if not q:
                    return
                n = 1
                if slots_left > 0 and len(q) > slots_left:
                    n = -(-len(q) // slots_left)
                for _ in range(min(n, len(q))):
                    if fillq:
                        fq_pop()
                    else:
                        side.pop(0)()

            # ---- attention machinery -------------------------------------
            pending = []      # (due_slot, thunk) attn@V
            pmuls = []        # deferred normalize multiplies
            proj_release = {}  # slot -> [sb...]

            def flush_due(slot):
                i = 0
                while i < len(pending):
                    if pending[i][0] <= slot:
                        pending.pop(i)[1]()
                    else:
                        i += 1

            def emit_norm(vq, qb, g2, po, tail=False):
                # per-h chains so h0's round trip never waits on h1; in the
                # tail h1's DMA hops ride the ACT hwdge queue in parallel.
                oc = [sbR.tile([65, 512], f32, tag="ocopy",
                               name=f"oc{vq}_{h}") for h in range(2)]
                tp = sbR.tile([P, 8], f32, tag="tp", name=f"tp{vq}")
                rp = sbR.tile([P, 8], f32, tag="rp", name=f"rp{vq}")
                bcs = []
                for h in range(2):
                    dmae = nc.scalar if (tail and h == 1) else nc.sync
                    if h == 0:
                        nc.scalar.copy(oc[h][:], po[h][:])
                    else:
                        nc.vector.tensor_copy(oc[h][:], po[h][:])
                    dmae.dma_start(out=tp[:, 4 * h : 4 * h + 4],
                                   in_=oc[h][64:65, :])
                    nc.vector.reciprocal(rp[:, 4 * h : 4 * h + 4],
                                         tp[:, 4 * h : 4 * h + 4])
                    rd = drS.tile([1, 512], f32, tag="rcd",
                                  name=f"rd{vq}_{h}")
                    dmae.dma_start(out=rd[:], in_=rp[:, 4 * h : 4 * h + 4])
                    bc = sbR.tile([64, 512], f32, tag="bcast",
                                  name=f"bc{vq}_{h}")
                    dmae.dma_start(out=bc[:],
                                   in_=rd[:].partition_broadcast(64))
                    bcs.append(bc)
                def muls():
                    # h0 on GPSIMD (idle; a stall on the bc round trip does
                    # not block the DVE queue), h1 on DVE in the tail.
                    for h in range(2):
                        eng = nc.vector if (tail and h == 1) else nc.gpsimd
                        eng.tensor_mul(
                            outTi[h * 64 : (h + 1) * 64, g2,
                                  qb * 512 : (qb + 1) * 512],
                            oc[h][0:64, :], bcs[h][:],
                        )
                pmuls.append((vq, muls))

            slot_of = lambda vq, kb: vq * NKB + kb

            # ---- prologue: minimal chains for vq=0 -----------------------
            # q00/k00 run on the (idle) psS ring so they pipeline with each
            # other and with the v0/v1 chains on psA.
            qch = qk_chain_thunks(qT, wq_t, 0, 0, alloc=_psS_acc)
            kch = qk_chain_thunks(kT, wk_t, 0, 0, alloc=_psS_acc)
            vch0 = v_chain_thunks(0)
            vch1 = v_chain_thunks(1)
            for a, b in zip(qch, kch):
                a()
                b()
            for t in vch0:
                t()
            for t in vch1:
                t()

            order = [("v", 2), ("v", 3), ("k", (0, 1)), ("v", 4), ("v", 5),
                     ("v", 6), ("k", (0, 2)), ("v", 7), ("v", 8), ("v", 9),
                     ("v", 10), ("k", (0, 3)), ("v", 11), ("v", 12),
                     ("v", 13), ("q", (1, 0)), ("k", (1, 0)), ("v", 14),
                     ("v", 15), ("k", (1, 1)), ("k", (1, 2)), ("k", (1, 3)),
                     ("q", (0, 1)), ("q", (1, 1)), ("q", (0, 2)),
                     ("q", (1, 2)), ("q", (0, 3)), ("q", (1, 3))]
            for kind, idx in order:
                if kind == "v":
                    fq_push("v", idx, v_chain_thunks(idx))
                elif kind == "k":
                    fq_push("k", idx, qk_chain_thunks(kT, wk_t, *idx))
                else:
                    fq_push("q", idx, qk_chain_thunks(qT, wq_t, *idx))

            for vq in range(NVQ):
                g2, qb = vq & 1, vq >> 1
                po = [psO.tile([65, 512], f32, tag="o",
                               name=f"po{vq}_{h}") for h in range(2)]
                at_tiles = {}
                for kb in range(NKB):
                    slot = slot_of(vq, kb)
                    kbp, t = kb >> 1, kb & 1
                    # gate: this slot's scores need kT block kb//4
                    drain_until(lambda: k_done(g2, kb // 4))
                    if t == 0:
                        at_tiles[kbp] = sbA.tile([P, 2, 2, 512], fp8,
                                                 tag="attnT",
                                                 name=f"at{vq}_{kbp}")
                    ps = psS.tile([P, 2, 512], f32, tag="s",
                                  name=f"ps{vq}_{kb}")
                    for hl in range(2):
                        nc.tensor.matmul(
                            ps[:, hl, :],
                            kT[g2][hl * 64 : (hl + 1) * 64,
                                   kb * P : (kb + 1) * P],
                            qT[g2][hl * 64 : (hl + 1) * 64,
                                   qb * 512 : (qb + 1) * 512],
                            start=True, stop=True,
                            tile_position=(hl * 64, 0),
                        )
                    at = at_tiles[kbp]
                    if kb in ACT_KBS:
                        nc.scalar.activation(
                            at[:, :, t, :], ps[:],
                            mybir.ActivationFunctionType.Exp,
                            scale=0.125, bias=bias_t[:],
                        )
                    else:
                        nc.vector.tensor_scalar(
                            out=at[:, :, t, :].bitcast(u8), in0=ps[:],
                            scalar1=SCH_MULT, scalar2=SCH_ADD,
                            op0=mybir.AluOpType.mult,
                            op1=mybir.AluOpType.add,
                        )
                    if t == 1:
                        for h in range(2):
                            def attnv(h=h, kbp=kbp, po=po, at=at, g2=g2):
                                nc.tensor.matmul(
                                    po[h][:],
                                    vaug[:, kbp, g2 * 2 + h, :, 0:65],
                                    at[:, h, :, :],
                                    start=(kbp == 0), stop=(kbp == NKBP - 1),
                                    perf_mode=DR,
                                )
                            pending.append((slot + 1 + 2 * h, attnv))
                    flush_due(slot)
                    if kb == 4:
                        for _ in range(len(pmuls)):
                            pmuls.pop(0)[1]()
                    if kb == 10:
                        # released well after the kb==4 muls flush so the
                        # gpsimd multiplies finish before proj MMs need outTi
                        for sb in proj_release.pop(vq, []):
                            side.extend(proj_thunks(sb))
                    # lookahead requirements
                    if vq == 0:
                        drain_until(
                            lambda: fill_state["v"] >= min(kb + 3, NKB - 1))
                    if kb >= 10:
                        nvq = vq + 1
                        if nvq < NVQ:
                            ng2, nqb = nvq & 1, nvq >> 1
                            drain_until(
                                lambda: (ng2, nqb) in fill_state["q"]
                                and k_done(ng2, 0))
                    if kb >= 13 and vq <= 1:
                        drain_until(lambda: k_done(vq & 1 ^ 1, 1))
                    slots_left = (NVQ - 1 - vq) * NKB + (NKB - 1 - kb)
                    pull_side(slots_left)
                # end of vq: flush all pending attn@V, then normalize
                flush_due(10 ** 9)
                emit_norm(vq, qb, g2, po, tail=(vq == NVQ - 1))
                if g2 == 1:
                    # proj for token range of qb once its muls flush (vq+1,kb4)
                    sbs = [4 * qb + i for i in range(4)]
                    if vq >= NVQ - 3:
                        # qb=2 and qb=3 projections are emitted in the tail:
                        # qb=2's are ready and overlap the final norm round
                        # trip, qb=3's follow once vq7's muls land.
                        pass
                    else:
                        proj_release[vq + 1] = sbs

            # ---- tail ----------------------------------------------------
            while fillq:
                fq_pop()
            # vq7 muls first (gpsimd; waits on the bc round trip there),
            # then the reserved qb=2 projections run during that round trip,
            # then the qb=3 projections once outTi is complete.
            for _ in range(len(pmuls)):
                pmuls.pop(0)[1]()
            while side:
                side.pop(0)()
            for sb in (8, 9, 10, 11, 12, 13, 14, 15):
                for t in proj_thunks(sb, tail=True):
                    t()

    _split_multi_waits(nc)
    return nc


# ---------------------------------------------------------------------------
# Host side
# ---------------------------------------------------------------------------
def make_in_maps(x, Wq, Wk, Wv, Wp):
    import ml_dtypes

    e4 = ml_dtypes.float8_e4m3
    x = np.ascontiguousarray(x, dtype=np.float32)
    Wq = np.asarray(Wq, dtype=np.float32)
    Wk = np.asarray(Wk, dtype=np.float32)
    Wv = np.asarray(Wv, dtype=np.float32)
    Wp = np.asarray(Wp, dtype=np.float32)

    def shuffle_w(w):  # [1024, 256] -> [128, 4, 2, 256]
        return np.ascontiguousarray(
            w.reshape(NC, 2, P, w.shape[1]).transpose(2, 0, 1, 3)
        ).astype(e4)

    def shuffle_w_qk(w):  # like shuffle_w, but cols h*64+o*32+p -> o*64+h*32+p
        w = w.reshape(1024, 2, 2, 2, 32).transpose(0, 1, 3, 2, 4)
        return shuffle_w(np.ascontiguousarray(w.reshape(1024, GCOLS)))

    in_maps = []
    xts = []
    for b in range(B):
        xT = x[b].T  # [1024, 2048]
        xts.append(np.ascontiguousarray(
            xT.reshape(NC, 2, P, N).transpose(2, 0, 1, 3)).astype(e4))
    for c in range(NCORES):
        b, g = divmod(c, 4)
        cs = slice(g * GCOLS, (g + 1) * GCOLS)
        wpi = np.ascontiguousarray(
            Wp[cs, :].reshape(2, P, D).transpose(1, 0, 2)).astype(e4)
        in_maps.append(
            {
                "xti": xts[b],
                "wqi": shuffle_w(Wq[:, cs]),
                "wki": shuffle_w(Wk[:, cs]),
                "wvi": shuffle_w(Wv[:, cs]),
                "wpi": wpi,
            }
        )
    return in_maps


def _gather(res, x):
    out = np.empty((B, N, D), dtype=np.float32)
    for b in range(B):
        acc = x[b].copy()
        for g in range(4):
            acc += res.results[b * 4 + g]["pout"].astype(np.float32)
        out[b] = acc
    return out


def kernel(x, Wq, Wk, Wv, Wp):
    global _last_results
    from concourse.bass_utils import run_bass_kernel_spmd

    x = np.ascontiguousarray(x, dtype=np.float32)

    if "nc" not in _cache:
        _cache["nc"] = _build()
    nc = _cache["nc"]

    in_maps = make_in_maps(x, Wq, Wk, Wv, Wp)
    # Transient device glitches (~rare) corrupt a run end-to-end; run twice
    # and only trust agreeing executions (third run breaks a disagreement).
    outs = []
    for _ in range(3):
        res = run_bass_kernel_spmd(nc, in_maps, core_ids=list(range(NCORES)))
        _last_results = res
        outs.append(_gather(res, x))
        if len(outs) >= 2:
            for a in range(len(outs) - 1):
                if np.allclose(outs[a], outs[-1], rtol=0, atol=2e-3):
                    return outs[-1]
    return outs[-1]
